# revision 40
# baseline (speedup 1.0000x reference)
"""Trainium2 Bass kernel for nn_ContrastiveLoss_66030827208766.

Strategy (data-parallel over images, captions replicated):
  - 8 cores, 16 images each. Captions (both caption sets) replicated.
  - Host prep: replicate-pad invalid objects (object 0) so max over all O
    objects == max over valid objects; pre-transpose all operands to
    D-major layout [D, cols]; precompute 1/caption_len.
  - Device per core: for each caption word w (50 + 30 chunks of 128
    captions), matmul capT[:, w-chunk] (stationary, [D,128]) against the
    image-object matrix (moving, [D, 16*O]) -> PSUM [128 caps, cols];
    segmented reduce_max over each image's O-block -> buf[c, i, w];
    one reduce_sum over w per part; combine with 1/len scalars ->
    scores_t [128 caps, 16 imgs] -> DRAM.
  - Host: gather 8 tiles -> full 128x128 score matrix -> triplet margin
    loss reduction (tiny) in numpy.
"""

import numpy as np

import concourse.bass as bass
import concourse.mybir as mybir
from concourse import tile
from concourse.bass_utils import run_bass_kernel_spmd
from concourse.tile_rust import add_dep_helper

B = 128          # batch (images == captions)
O1, W1 = 36, 50  # part 1: im objects, s words
O2, W2 = 25, 30  # part 2: pred objects, c_r words
D = 128
NCORES = 8
IPC = B // NCORES  # images per core
MARGIN = 0.2
F32 = mybir.dt.float32

# caption columns per resident SBUF tile (holds 5 w-chunks of 128 captions)
CAPCOLS = 640

LAST_RESULT = None  # BassKernelResults of the most recent run (for test.py)
_NC = None          # cached compiled program (shapes are static)


def _build_nc():
    nc = bass.Bass()
    capT1 = nc.dram_tensor("capT1", [D, B * W1], F32, kind="ExternalInput")
    capT2 = nc.dram_tensor("capT2", [D, B * W2], F32, kind="ExternalInput")
    imT1 = nc.dram_tensor("imT1", [D, IPC * O1], F32, kind="ExternalInput")
    imT2 = nc.dram_tensor("imT2", [D, IPC * O2], F32, kind="ExternalInput")
    rblob = nc.dram_tensor("rblob", [B, 2], F32, kind="ExternalInput")
    out_t = nc.dram_tensor("scores_t", [B, IPC], F32, kind="ExternalOutput")

    H1 = IPC * O1 // 2  # 288: half of part-1 image-object columns (8 images)

    with tile.TileContext(nc) as tc:
        with (
            tc.tile_pool(name="const", bufs=1) as cpool,
            tc.tile_pool(name="psum", bufs=1, space="PSUM") as pspool,
            tc.tile_pool(name="work", bufs=1) as wpool,
        ):
            # PE matmuls can carry at most ONE sync wait in codegen.  Each
            # freshly-DMA'd tile is first touched by a degenerate 1x1
            # "junk" matmul whose only real dep is that DMA; once the PE
            # has observed the DMA's queue tick there, the real matmuls'
            # identical requirement is elided and they keep only their
            # PSUM WAR wait.
            hoisted = {}
            pending_nops = []

            def hoist(key, corner_ap):
                if key in hoisted:
                    return
                hoisted[key] = nc.tensor.matmul(
                    junk_ps[:, :], corner_ap, corner_ap,
                    start=True, stop=True, skip_group_check=True,
                )
                pending_nops.append(hoisted[key])

            def order_after_nops(mm):
                # Pin the absorbing junk matmuls ahead of this matmul in
                # the PE stream (no-sync edge) so its waits are already
                # observed by the engine when it issues.
                while pending_nops:
                    add_dep_helper(mm.ins, pending_nops.pop().ins, sync=False,
                                   reason="order matmul after wait-carrier")

            # Exactly 7 input DMAs + 1 output DMA: queue assignment is
            # global round-robin over 8 HWDGE queues, so every DMA gets
            # its own queue — no same-queue FIFO waits anywhere.
            CW1 = B * W1 // 2  # 3200 columns per cap1 half (25 w-chunks)
            CW2 = B * W2 // 2  # 1920 columns per cap2 half (15 w-chunks)
            cap1_tiles = []
            for j in range(2):
                t = cpool.tile([D, CW1], F32, tag=f"cap1_{j}")
                nc.sync.dma_start(t[:], capT1[:, j * CW1:(j + 1) * CW1])
                cap1_tiles.append(t)
            cap2_tiles = []
            for j in range(2):
                t = cpool.tile([D, CW2], F32, tag=f"cap2_{j}")
                nc.sync.dma_start(t[:], capT2[:, j * CW2:(j + 1) * CW2])
                cap2_tiles.append(t)
            imt1 = cpool.tile([D, IPC * O1], F32, tag="imt1")
            nc.sync.dma_start(imt1[:], imT1[:])
            imt2 = cpool.tile([D, IPC * O2], F32, tag="imt2")
            nc.sync.dma_start(imt2[:], imT2[:])
            rblob_sb = cpool.tile([B, 2], F32, tag="rblob")
            nc.sync.dma_start(rblob_sb[:], rblob[:])
            r1 = rblob_sb[:, 0:1]
            r2 = rblob_sb[:, 1:2]

            # w-major so each step's reduce writes a contiguous, disjoint
            # range (otherwise Tile sees overlapping write bounding boxes
            # and chains same-engine WAW sem waits onto every reduce).
            buf1 = wpool.tile([B, W1, IPC], F32, tag="buf1")
            buf2 = wpool.tile([B, W2, IPC], F32, tag="buf2")

            # Static full-bank ping-pong PSUM tiles (avoid pool slot
            # rotation: its release mechanism bundles both accessor
            # engines' waits onto the allocating matmul, which exceeds
            # the 1-sync-wait budget of fp32 Matmult in codegen).
            NPS = 4
            ps_tiles = [pspool.tile([B, 512], F32, tag=f"ps{i}",
                                    name=f"ps{i}")
                        for i in range(NPS)]
            junk_ps = pspool.tile([1, 1], F32, tag="junk_ps", name="junk_ps")

            # Ping-pong over NPS static psum tiles.  The matmul reusing a
            # tile has a WAR dep on the reduce NPS steps back; absorb it
            # on a PE nop so the matmul itself carries at most one wait.
            reads = []  # reduce instruction per step

            def next_ps(ncols):
                k = len(reads)
                return ps_tiles[k % NPS][:, :ncols]

            for w in range(W1):
                hoist(("cap1", w // 25), cap1_tiles[w // 25][:1, :1])
                hoist(("imt1",), imt1[:1, :1])
                cs = cap1_tiles[w // 25][:, (w % 25) * B:(w % 25) * B + B]
                for h in range(2):
                    ps = next_ps(H1)
                    mm = nc.tensor.matmul(
                        ps, cs, imt1[:, h * H1:(h + 1) * H1],
                        start=True, stop=True,
                    )
                    order_after_nops(mm)
                    red = nc.vector.reduce_max(
                        buf1[:, w, h * (IPC // 2):(h + 1) * (IPC // 2)],
                        ps.rearrange("p (g o) -> p g o", o=O1),
                        axis=mybir.AxisListType.X,
                    )
                    reads.append(red)
            for w in range(W2):
                hoist(("cap2", w // 15), cap2_tiles[w // 15][:1, :1])
                hoist(("imt2",), imt2[:1, :1])
                cs = cap2_tiles[w // 15][:, (w % 15) * B:(w % 15) * B + B]
                ps = next_ps(IPC * O2)
                mm = nc.tensor.matmul(
                    ps, cs, imt2[:], start=True, stop=True,
                )
                order_after_nops(mm)
                red = nc.vector.reduce_max(
                    buf2[:, w, :],
                    ps.rearrange("p (g o) -> p g o", o=O2),
                    axis=mybir.AxisListType.X,
                )
                reads.append(red)

            s1 = wpool.tile([B, IPC], F32, tag="s1")
            s2 = wpool.tile([B, IPC], F32, tag="s2")
            nc.vector.reduce_sum(s1[:], buf1[:].rearrange("p w i -> p i w"),
                                 axis=mybir.AxisListType.X)
            nc.vector.reduce_sum(s2[:], buf2[:].rearrange("p w i -> p i w"),
                                 axis=mybir.AxisListType.X)
            nc.vector.tensor_scalar_mul(s2[:], s2[:], r2)
            # s1 = s1 * r1 + s2  (fused per-partition scalar + tensor op)
            nc.vector.scalar_tensor_tensor(
                s1[:], s1[:], r1, s2[:],
                op0=mybir.AluOpType.mult, op1=mybir.AluOpType.add,
            )
            out_dma = nc.sync.dma_start(out_t[:], s1[:])

    # Strip always-satisfied own-engine waits from DVE instructions: the
    # DVE drains its pipe between ops, so a DVE instruction's wait on an
    # earlier DVE instruction's completion sem is satisfied at issue.
    # Tile emits these redundant waits (previous-reader chains on reused
    # psum tiles) and they exceed walrus' 1-sync-wait budget for
    # TensorReduce.
    # The kernel-tail drain waits on every DMA queue touched, exceeding
    # the per-instruction wait budget.  Input-DMA completions are covered
    # transitively (each input is read by compute gated on it, and the
    # drain waits on the engines' final ticks), so only the output DMA's
    # queue wait is load-bearing.
    out_q = {u.ant_name for u in out_dma.ins.sync_info.on_update
             if u.ant_name.startswith("DMAHW")}

    # Same story for HWDGE DMAs: per-queue FIFO makes a DMA's wait on its
    # own queue's earlier completions redundant.
    for bb in nc.main_func.blocks:
        for ins in bb.instructions:
            si = ins.sync_info
            if si is None:
                continue
            if type(ins).__name__ == "InstDrain" and len(si.on_wait) > 2:
                # Keep only the output DMA's queue wait: engine completion
                # is enforced by the per-engine drains + EVSEM butterfly
                # that follow, and input-DMA completions are covered
                # transitively by the compute that consumed them.
                drop = lambda w: w.ant_name not in out_q
            elif type(ins).__name__ == "InstMatmult":
                # WAW on a reused psum bank: with NPS=4 rotation the prior
                # matmul's drain (~128 cyc) finished >=3 matmul-streams
                # ago, so the same-engine completion wait is dead.
                drop = lambda w: w.ant_name.startswith("PE_")
            elif getattr(ins, "engine", None) == mybir.EngineType.DVE:
                drop = lambda w: w.ant_name.startswith("DVE_")
            elif type(ins).__name__ == "InstDMACopy":
                own = {u.ant_name for u in si.on_update
                       if u.ant_name.startswith("DMAHW")}
                drop = lambda w: w.ant_name in own
            else:
                continue
            kept = [w for w in si.on_wait if not drop(w)]
            if len(kept) != len(si.on_wait):
                si.on_wait = kept
                ins.sync_info = si
    return nc


def _pad_objects(x, lens):
    """Replace invalid object rows (o >= lens[i]) with object 0 (always
    valid) so a plain max over all objects equals the masked max."""
    ocount = x.shape[1]
    valid = np.arange(ocount)[None, :, None] < lens[:, None, None]
    return np.where(valid, x, x[:, :1, :])


def _dmajor(x):
    """(B, W, D) -> [D, W*B] with column index w*B + c."""
    b, w, d = x.shape
    return np.ascontiguousarray(x.transpose(1, 0, 2).reshape(w * b, d).T)


def kernel(im, im_l, s, s_l, pred, pred_l, cap_o_pred, cap_o_l, c_r_pred,
           c_r_l, trace=False, tmpdir=None):
    global LAST_RESULT, _NC
    im = np.asarray(im, dtype=np.float32)
    s = np.asarray(s, dtype=np.float32)
    pred = np.asarray(pred, dtype=np.float32)
    c_r_pred = np.asarray(c_r_pred, dtype=np.float32)
    im_l = np.asarray(im_l)
    pred_l = np.asarray(pred_l)

    im_p = _pad_objects(im, im_l)
    pred_p = _pad_objects(pred, pred_l)

    capT1 = _dmajor(s)
    capT2 = _dmajor(c_r_pred)
    rblob = np.stack([1.0 / np.asarray(s_l, dtype=np.float32),
                      1.0 / np.asarray(c_r_l, dtype=np.float32)], axis=1)

    in_maps = []
    for m in range(NCORES):
        sl = slice(m * IPC, (m + 1) * IPC)
        in_maps.append({
            "capT1": capT1,
            "capT2": capT2,
            "imT1": np.ascontiguousarray(
                im_p[sl].reshape(IPC * O1, D).T),
            "imT2": np.ascontiguousarray(
                pred_p[sl].reshape(IPC * O2, D).T),
            "rblob": rblob,
        })

    if _NC is None:
        _NC = _build_nc()
    res = run_bass_kernel_spmd(_NC, in_maps, list(range(NCORES)), trace=trace,
                               tmpdir=tmpdir)
    LAST_RESULT = res

    # scores_t[m] is [128 captions, 16 images] for images m*16..m*16+15.
    scores = np.concatenate(
        [res.results[m]["scores_t"].T for m in range(NCORES)], axis=0)

    # Triplet margin loss on the full (tiny) B x B matrix.
    d = np.diag(scores).copy()
    cost_s = np.maximum(MARGIN + scores - d[:, None], 0.0).astype(np.float32)
    cost_im = np.maximum(MARGIN + scores - d[None, :], 0.0).astype(np.float32)
    np.fill_diagonal(cost_s, 0.0)
    np.fill_diagonal(cost_im, 0.0)
    out = cost_s.max(axis=1).sum() + cost_im.max(axis=0).sum()
    return np.asarray(out, dtype=np.float32)


# revision 41
# speedup vs baseline: 1.2088x; 1.2088x over previous
"""Trainium2 Bass kernel for nn_ContrastiveLoss_66030827208766.

Strategy (data-parallel over images, captions replicated):
  - 8 cores, 16 images each. Captions (both caption sets) replicated.
  - Host prep: replicate-pad invalid objects (object 0) so max over all O
    objects == max over valid objects; pre-transpose all operands to
    D-major layout [D, cols]; precompute 1/caption_len.
  - Device per core: for each caption word w (50 + 30 chunks of 128
    captions), matmul capT[:, w-chunk] (stationary, [D,128]) against the
    image-object matrix (moving, [D, 16*O]) -> PSUM [128 caps, cols];
    segmented reduce_max over each image's O-block -> buf[c, i, w];
    one reduce_sum over w per part; combine with 1/len scalars ->
    scores_t [128 caps, 16 imgs] -> DRAM.
  - Host: gather 8 tiles -> full 128x128 score matrix -> triplet margin
    loss reduction (tiny) in numpy.
"""

import numpy as np

import concourse.bass as bass
import concourse.mybir as mybir
from concourse import tile
from concourse.bass_utils import run_bass_kernel_spmd
from concourse.tile_rust import add_dep_helper

B = 128          # batch (images == captions)
O1, W1 = 36, 50  # part 1: im objects, s words
O2, W2 = 25, 30  # part 2: pred objects, c_r words
D = 128
NCORES = 8
IPC = B // NCORES  # images per core
MARGIN = 0.2
F32 = mybir.dt.float32
F32R = mybir.dt.float32r

# caption columns per resident SBUF tile (holds 5 w-chunks of 128 captions)
CAPCOLS = 640

LAST_RESULT = None  # BassKernelResults of the most recent run (for test.py)
_NC = None          # cached compiled program (shapes are static)


def _build_nc():
    nc = bass.Bass()
    capT1 = nc.dram_tensor("capT1", [D, B * W1], F32R, kind="ExternalInput")
    capT2 = nc.dram_tensor("capT2", [D, B * W2], F32R, kind="ExternalInput")
    imT1 = nc.dram_tensor("imT1", [D, IPC * O1], F32R, kind="ExternalInput")
    imT2 = nc.dram_tensor("imT2", [D, IPC * O2], F32R, kind="ExternalInput")
    rblob = nc.dram_tensor("rblob", [B, 2], F32, kind="ExternalInput")
    out_t = nc.dram_tensor("scores_t", [B, IPC], F32, kind="ExternalOutput")

    H1 = IPC * O1 // 2  # 288: half of part-1 image-object columns (8 images)

    with tile.TileContext(nc) as tc:
        with (
            tc.tile_pool(name="const", bufs=1) as cpool,
            tc.tile_pool(name="psum", bufs=1, space="PSUM") as pspool,
            tc.tile_pool(name="work", bufs=1) as wpool,
        ):
            # PE matmuls can carry at most ONE sync wait in codegen.  Each
            # freshly-DMA'd tile is first touched by a degenerate 1x1
            # "junk" matmul whose only real dep is that DMA; once the PE
            # has observed the DMA's queue tick there, the real matmuls'
            # identical requirement is elided and they keep only their
            # PSUM WAR wait.
            hoisted = {}
            pending_nops = []

            def hoist(key, corner_ap):
                if key in hoisted:
                    return
                corner_ap = corner_ap.bitcast(F32)
                hoisted[key] = nc.tensor.matmul(
                    junk_ps[:, :], corner_ap, corner_ap,
                    start=True, stop=True, skip_group_check=True,
                )
                pending_nops.append(hoisted[key])

            def order_after_nops(mm):
                # Pin the absorbing junk matmuls ahead of this matmul in
                # the PE stream (no-sync edge) so its waits are already
                # observed by the engine when it issues.
                while pending_nops:
                    add_dep_helper(mm.ins, pending_nops.pop().ins, sync=False,
                                   reason="order matmul after wait-carrier")

            # Exactly 7 input DMAs + 1 output DMA: queue assignment is
            # global round-robin over 8 HWDGE queues, so every DMA gets
            # its own queue — no same-queue FIFO waits anywhere.
            CW1 = B * W1 // 2  # 3200 columns per cap1 half (25 w-chunks)
            CW2 = B * W2 // 2  # 1920 columns per cap2 half (15 w-chunks)
            cap1_tiles = []
            for j in range(2):
                t = cpool.tile([D, CW1], F32R, tag=f"cap1_{j}")
                nc.sync.dma_start(t[:], capT1[:, j * CW1:(j + 1) * CW1])
                cap1_tiles.append(t)
            cap2_tiles = []
            for j in range(2):
                t = cpool.tile([D, CW2], F32R, tag=f"cap2_{j}")
                nc.sync.dma_start(t[:], capT2[:, j * CW2:(j + 1) * CW2])
                cap2_tiles.append(t)
            imt1 = cpool.tile([D, IPC * O1], F32R, tag="imt1")
            nc.sync.dma_start(imt1[:], imT1[:])
            imt2 = cpool.tile([D, IPC * O2], F32R, tag="imt2")
            nc.sync.dma_start(imt2[:], imT2[:])
            rblob_sb = cpool.tile([B, 2], F32, tag="rblob")
            nc.sync.dma_start(rblob_sb[:], rblob[:])
            r1 = rblob_sb[:, 0:1]
            r2 = rblob_sb[:, 1:2]

            # w-major so each step's reduce writes a contiguous, disjoint
            # range (otherwise Tile sees overlapping write bounding boxes
            # and chains same-engine WAW sem waits onto every reduce).
            buf1 = wpool.tile([B, W1, IPC], F32, tag="buf1")
            buf2 = wpool.tile([B, W2, IPC], F32, tag="buf2")

            # Static full-bank ping-pong PSUM tiles (avoid pool slot
            # rotation: its release mechanism bundles both accessor
            # engines' waits onto the allocating matmul, which exceeds
            # the 1-sync-wait budget of fp32 Matmult in codegen).
            NPS = 4
            ps_tiles = [pspool.tile([B, 512], F32, tag=f"ps{i}",
                                    name=f"ps{i}")
                        for i in range(NPS)]
            junk_ps = pspool.tile([1, 1], F32, tag="junk_ps", name="junk_ps")

            # Ping-pong over NPS static psum tiles.  The matmul reusing a
            # tile has a WAR dep on the reduce NPS steps back; absorb it
            # on a PE nop so the matmul itself carries at most one wait.
            reads = []  # reduce instruction per step

            def next_ps(ncols):
                k = len(reads)
                return ps_tiles[k % NPS][:, :ncols]

            for w in range(W1):
                hoist(("cap1", w // 25), cap1_tiles[w // 25][:1, :1])
                hoist(("imt1",), imt1[:1, :1])
                cs = cap1_tiles[w // 25][:, (w % 25) * B:(w % 25) * B + B]
                for h in range(2):
                    ps = next_ps(H1)
                    mm = nc.tensor.matmul(
                        ps, cs, imt1[:, h * H1:(h + 1) * H1],
                        start=True, stop=True,
                    )
                    order_after_nops(mm)
                    red = nc.vector.reduce_max(
                        buf1[:, w, h * (IPC // 2):(h + 1) * (IPC // 2)],
                        ps.rearrange("p (g o) -> p g o", o=O1),
                        axis=mybir.AxisListType.X,
                    )
                    reads.append(red)
            for w in range(W2):
                hoist(("cap2", w // 15), cap2_tiles[w // 15][:1, :1])
                hoist(("imt2",), imt2[:1, :1])
                cs = cap2_tiles[w // 15][:, (w % 15) * B:(w % 15) * B + B]
                ps = next_ps(IPC * O2)
                mm = nc.tensor.matmul(
                    ps, cs, imt2[:], start=True, stop=True,
                )
                order_after_nops(mm)
                red = nc.vector.reduce_max(
                    buf2[:, w, :],
                    ps.rearrange("p (g o) -> p g o", o=O2),
                    axis=mybir.AxisListType.X,
                )
                reads.append(red)

            s1 = wpool.tile([B, IPC], F32, tag="s1")
            s2 = wpool.tile([B, IPC], F32, tag="s2")
            nc.vector.reduce_sum(s1[:], buf1[:].rearrange("p w i -> p i w"),
                                 axis=mybir.AxisListType.X)
            nc.vector.reduce_sum(s2[:], buf2[:].rearrange("p w i -> p i w"),
                                 axis=mybir.AxisListType.X)
            nc.vector.tensor_scalar_mul(s2[:], s2[:], r2)
            # s1 = s1 * r1 + s2  (fused per-partition scalar + tensor op)
            nc.vector.scalar_tensor_tensor(
                s1[:], s1[:], r1, s2[:],
                op0=mybir.AluOpType.mult, op1=mybir.AluOpType.add,
            )
            out_dma = nc.sync.dma_start(out_t[:], s1[:])

    # Strip always-satisfied own-engine waits from DVE instructions: the
    # DVE drains its pipe between ops, so a DVE instruction's wait on an
    # earlier DVE instruction's completion sem is satisfied at issue.
    # Tile emits these redundant waits (previous-reader chains on reused
    # psum tiles) and they exceed walrus' 1-sync-wait budget for
    # TensorReduce.
    # The kernel-tail drain waits on every DMA queue touched, exceeding
    # the per-instruction wait budget.  Input-DMA completions are covered
    # transitively (each input is read by compute gated on it, and the
    # drain waits on the engines' final ticks), so only the output DMA's
    # queue wait is load-bearing.
    out_q = {u.ant_name for u in out_dma.ins.sync_info.on_update
             if u.ant_name.startswith("DMAHW")}

    # Same story for HWDGE DMAs: per-queue FIFO makes a DMA's wait on its
    # own queue's earlier completions redundant.
    for bb in nc.main_func.blocks:
        for ins in bb.instructions:
            si = ins.sync_info
            if si is None:
                continue
            if type(ins).__name__ == "InstDrain" and len(si.on_wait) > 2:
                # Keep only the output DMA's queue wait: engine completion
                # is enforced by the per-engine drains + EVSEM butterfly
                # that follow, and input-DMA completions are covered
                # transitively by the compute that consumed them.
                drop = lambda w: w.ant_name not in out_q
            elif type(ins).__name__ == "InstMatmult":
                # WAW on a reused psum bank: with NPS=4 rotation the prior
                # matmul's drain (~128 cyc) finished >=3 matmul-streams
                # ago, so the same-engine completion wait is dead.
                drop = lambda w: w.ant_name.startswith("PE_")
            elif getattr(ins, "engine", None) == mybir.EngineType.DVE:
                drop = lambda w: w.ant_name.startswith("DVE_")
            elif type(ins).__name__ == "InstDMACopy":
                own = {u.ant_name for u in si.on_update
                       if u.ant_name.startswith("DMAHW")}
                drop = lambda w: w.ant_name in own
            else:
                continue
            kept = [w for w in si.on_wait if not drop(w)]
            if len(kept) != len(si.on_wait):
                si.on_wait = kept
                ins.sync_info = si
    return nc


def _pad_objects(x, lens):
    """Replace invalid object rows (o >= lens[i]) with object 0 (always
    valid) so a plain max over all objects equals the masked max."""
    ocount = x.shape[1]
    valid = np.arange(ocount)[None, :, None] < lens[:, None, None]
    return np.where(valid, x, x[:, :1, :])


def _dmajor(x):
    """(B, W, D) -> [D, W*B] with column index w*B + c."""
    b, w, d = x.shape
    return np.ascontiguousarray(x.transpose(1, 0, 2).reshape(w * b, d).T)


def _tf32_round(x):
    """Round fp32 to TF32 (10-bit mantissa), the float32r input format.
    The PE accumulates in fp32, so pre-rounded inputs lose nothing more."""
    b = x.astype(np.float32).view(np.uint32).astype(np.uint64)
    r = b + 0xFFF + ((b >> 13) & 1)
    r = (r & ~np.uint64(0x1FFF)).astype(np.uint32)
    return r.view(np.float32)


def kernel(im, im_l, s, s_l, pred, pred_l, cap_o_pred, cap_o_l, c_r_pred,
           c_r_l, trace=False, tmpdir=None):
    global LAST_RESULT, _NC
    im = np.asarray(im, dtype=np.float32)
    s = np.asarray(s, dtype=np.float32)
    pred = np.asarray(pred, dtype=np.float32)
    c_r_pred = np.asarray(c_r_pred, dtype=np.float32)
    im_l = np.asarray(im_l)
    pred_l = np.asarray(pred_l)

    im_p = _tf32_round(_pad_objects(im, im_l))
    pred_p = _tf32_round(_pad_objects(pred, pred_l))

    capT1 = _dmajor(_tf32_round(s))
    capT2 = _dmajor(_tf32_round(c_r_pred))
    rblob = np.stack([1.0 / np.asarray(s_l, dtype=np.float32),
                      1.0 / np.asarray(c_r_l, dtype=np.float32)], axis=1)

    in_maps = []
    for m in range(NCORES):
        sl = slice(m * IPC, (m + 1) * IPC)
        in_maps.append({
            "capT1": capT1,
            "capT2": capT2,
            "imT1": np.ascontiguousarray(
                im_p[sl].reshape(IPC * O1, D).T),
            "imT2": np.ascontiguousarray(
                pred_p[sl].reshape(IPC * O2, D).T),
            "rblob": rblob,
        })

    if _NC is None:
        _NC = _build_nc()
    res = run_bass_kernel_spmd(_NC, in_maps, list(range(NCORES)), trace=trace,
                               tmpdir=tmpdir)
    LAST_RESULT = res

    # scores_t[m] is [128 captions, 16 images] for images m*16..m*16+15.
    scores = np.concatenate(
        [res.results[m]["scores_t"].T for m in range(NCORES)], axis=0)

    # Triplet margin loss on the full (tiny) B x B matrix.
    d = np.diag(scores).copy()
    cost_s = np.maximum(MARGIN + scores - d[:, None], 0.0).astype(np.float32)
    cost_im = np.maximum(MARGIN + scores - d[None, :], 0.0).astype(np.float32)
    np.fill_diagonal(cost_s, 0.0)
    np.fill_diagonal(cost_im, 0.0)
    out = cost_s.max(axis=1).sum() + cost_im.max(axis=0).sum()
    return np.asarray(out, dtype=np.float32)


# revision 42
# speedup vs baseline: 1.3286x; 1.0991x over previous
"""Trainium2 Bass kernel for nn_ContrastiveLoss_66030827208766.

Strategy (data-parallel over images, captions replicated):
  - 8 cores, 16 images each. Captions (both caption sets) replicated.
  - Host prep: replicate-pad invalid objects (object 0) so max over all O
    objects == max over valid objects; cast everything to bf16 (the PE
    accumulates in fp32; end-to-end loss error ~1e-5); pre-transpose all
    operands to D-major layout [D, cols].
  - Device per core: for each caption word w (50 + 30 chunks of 128
    captions), matmul capT[:, w-chunk] (stationary, [D,128]) against the
    image-object matrix (moving, [D, 16*O]) -> PSUM [128 caps, cols];
    segmented reduce_max over each image's O-block -> buf[c, w, i];
    one reduce_sum over w per part; combine with 1/len scalars ->
    scores_t [128 caps, 16 imgs] -> DRAM.
  - Host: gather 8 tiles -> full 128x128 score matrix -> triplet margin
    loss reduction (tiny) in numpy.

Codegen constraint: every TPB instruction can carry at most ONE sync
wait.  Three tactics keep us within it: (1) freshly-DMA'd tiles are
first touched by degenerate 1x1 "junk" matmuls so the real matmuls'
DMA-queue requirements are already observed by the PE; (2) buffers are
laid out so each writer hits a disjoint range (no spurious WAW chains);
(3) a post-pass strips waits that are redundant by construction
(same-engine in-order completion, per-queue DMA FIFO, barrier-covered
drain waits).
"""

import ml_dtypes
import numpy as np

import concourse.bass as bass
import concourse.mybir as mybir
from concourse import tile
from concourse.bass_utils import run_bass_kernel_spmd
from concourse.tile_rust import add_dep_helper

B = 128          # batch (images == captions)
O1, W1 = 36, 50  # part 1: im objects, s words
O2, W2 = 25, 30  # part 2: pred objects, c_r words
D = 128
NCORES = 8
IPC = B // NCORES  # images per core
MARGIN = 0.2
F32 = mybir.dt.float32
BF16 = mybir.dt.bfloat16

LAST_RESULT = None  # BassKernelResults of the most recent run (for test.py)
_NC = None          # cached program (shapes are static)


def _build_nc():
    nc = bass.Bass()
    capT1 = nc.dram_tensor("capT1", [D, B * W1], BF16, kind="ExternalInput")
    capT2 = nc.dram_tensor("capT2", [D, B * W2], BF16, kind="ExternalInput")
    imT1 = nc.dram_tensor("imT1", [D, IPC * O1], BF16, kind="ExternalInput")
    imT2 = nc.dram_tensor("imT2", [D, IPC * O2], BF16, kind="ExternalInput")
    rblob = nc.dram_tensor("rblob", [B, 2], F32, kind="ExternalInput")
    out_t = nc.dram_tensor("scores_t", [B, IPC], F32, kind="ExternalOutput")

    H1 = IPC * O1 // 2  # 288: half of part-1 image-object columns (8 images)

    with tile.TileContext(nc) as tc:
        with (
            tc.tile_pool(name="const", bufs=1) as cpool,
            tc.tile_pool(name="psum", bufs=1, space="PSUM") as pspool,
            tc.tile_pool(name="work", bufs=1) as wpool,
        ):
            # ---- input DMAs -------------------------------------------
            # Emission order fixes both the DMAHW bookkeeping lane
            # (global round-robin over 8) and the issuing engine
            # (alternating sync/scalar = the two physical HWDGE rings).
            # Strict alternation keeps every lane single-engine, so
            # per-lane FIFO order holds and same-lane waits are safely
            # strippable.  Pieces are emitted in first-use order so
            # compute starts as soon as the first pieces land.
            dma_idx = [0]

            def load(dst_ap, src_ap):
                eng = nc.sync if dma_idx[0] % 2 == 0 else nc.scalar
                dma_idx[0] += 1
                return eng.dma_start(dst_ap, src_ap)

            NP1 = 10  # cap1 pieces (5 w-chunks each)
            NP2 = 3   # cap2 pieces (10 w-chunks each)
            P1C = B * W1 // NP1
            P2C = B * W2 // NP2

            imt1 = cpool.tile([D, IPC * O1], BF16, tag="imt1")
            load(imt1[:], imT1[:])
            cap1 = cpool.tile([D, B * W1], BF16, tag="cap1")
            for j in range(NP1):
                load(cap1[:, j * P1C:(j + 1) * P1C],
                     capT1[:, j * P1C:(j + 1) * P1C])
            imt2 = cpool.tile([D, IPC * O2], BF16, tag="imt2")
            load(imt2[:], imT2[:])
            cap2 = cpool.tile([D, B * W2], BF16, tag="cap2")
            for j in range(NP2):
                load(cap2[:, j * P2C:(j + 1) * P2C],
                     capT2[:, j * P2C:(j + 1) * P2C])
            rblob_sb = cpool.tile([B, 2], F32, tag="rblob")
            load(rblob_sb[:], rblob[:])
            r1 = rblob_sb[:, 0:1]
            r2 = rblob_sb[:, 1:2]
            assert dma_idx[0] == 16, dma_idx

            # w-major so each step's reduce writes a contiguous, disjoint
            # range (otherwise Tile sees overlapping write bounding boxes
            # and chains same-engine WAW sem waits onto every reduce).
            buf1 = wpool.tile([B, W1, IPC], F32, tag="buf1")
            buf2 = wpool.tile([B, W2, IPC], F32, tag="buf2")

            # Static psum tiles (pool slot rotation would bundle both
            # accessor engines' release waits onto the allocating
            # matmul).  Part 1 uses two 2-bank tiles: both matmul halves
            # of one w land in one tile (bank 0 cols 0:288, bank 1 cols
            # 512:800) and a single 4D-strided reduce consumes them.
            ps1_tiles = [pspool.tile([B, 1024], F32, tag=f"ps1_{i}",
                                     name=f"ps1_{i}") for i in range(2)]
            ps2_tiles = [pspool.tile([B, 512], F32, tag=f"ps2_{i}",
                                     name=f"ps2_{i}") for i in range(2)]
            junk_ps = pspool.tile([1, 1], F32, tag="junk_ps", name="junk_ps")

            # 1x1 junk matmuls: first PE touch of each freshly-DMA'd tile.
            hoisted = {}
            pending = []

            def hoist(key, corner_ap):
                if key in hoisted:
                    return
                hoisted[key] = nc.tensor.matmul(
                    junk_ps[:, :], corner_ap, corner_ap,
                    start=True, stop=True, skip_group_check=True,
                )
                pending.append(hoisted[key])

            def order_after_pending(mm):
                while pending:
                    add_dep_helper(mm.ins, pending.pop().ins, sync=False,
                                   reason="order matmul after wait-carrier")

            for w in range(W1):
                hoist(("cap1", w // 5),
                      cap1[:1, (w // 5) * P1C:(w // 5) * P1C + 1])
                hoist(("imt1",), imt1[:1, :1])
                cs = cap1[:, w * B:(w + 1) * B]
                ps = ps1_tiles[w % 2]
                for h in range(2):
                    mm = nc.tensor.matmul(
                        ps[:, h * 512:h * 512 + H1], cs,
                        imt1[:, h * H1:(h + 1) * H1],
                        start=True, stop=True,
                    )
                    order_after_pending(mm)
                nc.vector.reduce_max(
                    buf1[:, w, :],
                    ps[:].rearrange("p (b x) -> p b x", b=2)[:, :, :H1]
                         .rearrange("p b (g o) -> p b g o", o=O1),
                    axis=mybir.AxisListType.X,
                )
            for w in range(W2):
                hoist(("cap2", w // 10),
                      cap2[:1, (w // 10) * P2C:(w // 10) * P2C + 1])
                hoist(("imt2",), imt2[:1, :1])
                cs = cap2[:, w * B:(w + 1) * B]
                ps = ps2_tiles[w % 2]
                mm = nc.tensor.matmul(
                    ps[:, :IPC * O2], cs, imt2[:],
                    start=True, stop=True,
                )
                order_after_pending(mm)
                nc.vector.reduce_max(
                    buf2[:, w, :],
                    ps[:, :IPC * O2].rearrange("p (g o) -> p g o", o=O2),
                    axis=mybir.AxisListType.X,
                )

            s1 = wpool.tile([B, IPC], F32, tag="s1")
            s2 = wpool.tile([B, IPC], F32, tag="s2")
            nc.vector.reduce_sum(s1[:], buf1[:].rearrange("p w i -> p i w"),
                                 axis=mybir.AxisListType.X)
            nc.vector.reduce_sum(s2[:], buf2[:].rearrange("p w i -> p i w"),
                                 axis=mybir.AxisListType.X)
            nc.vector.tensor_scalar_mul(s2[:], s2[:], r2)
            # s1 = s1 * r1 + s2  (fused per-partition scalar + tensor op)
            nc.vector.scalar_tensor_tensor(
                s1[:], s1[:], r1, s2[:],
                op0=mybir.AluOpType.mult, op1=mybir.AluOpType.add,
            )
            out_dma = nc.sync.dma_start(out_t[:], s1[:])

    # ---- wait-strip post-pass ----------------------------------------
    # Walrus codegen accepts at most one sync wait per instruction;
    # remove waits that are redundant by construction.
    out_q = {u.ant_name for u in out_dma.ins.sync_info.on_update
             if u.ant_name.startswith("DMAHW")}
    for bb in nc.main_func.blocks:
        for ins in bb.instructions:
            si = ins.sync_info
            if si is None:
                continue
            t = type(ins).__name__
            if t == "InstDrain" and len(si.on_wait) > 2:
                # Kernel-tail drain: engine completion is enforced by the
                # per-engine drains + EVSEM butterfly that follow, and
                # input-DMA completions are covered transitively by the
                # compute that consumed them.  Only the output DMA's
                # queue wait is load-bearing.
                drop = lambda w: w.ant_name not in out_q
            elif t == "InstMatmult":
                # WAW on a reused psum bank: the prior matmul's drain
                # (~128 cyc) finished >=2 matmul-streams (>=400 cyc)
                # earlier, so the same-engine completion wait is dead.
                drop = lambda w: w.ant_name.startswith("PE_")
            elif getattr(ins, "engine", None) == mybir.EngineType.DVE:
                # DVE fully drains its pipe between ops; waits on earlier
                # DVE completions are satisfied at issue.
                drop = lambda w: w.ant_name.startswith("DVE_")
            elif t == "InstDMACopy":
                # Per-lane FIFO (single issuing engine per lane by
                # construction) makes own-lane waits redundant.
                own = {u.ant_name for u in si.on_update
                       if u.ant_name.startswith("DMAHW")}
                drop = lambda w: w.ant_name in own
            else:
                continue
            kept = [w for w in si.on_wait if not drop(w)]
            if len(kept) != len(si.on_wait):
                si.on_wait = kept
                ins.sync_info = si
    return nc


def _pad_objects(x, lens):
    """Replace invalid object rows (o >= lens[i]) with object 0 (always
    valid) so a plain max over all objects equals the masked max."""
    ocount = x.shape[1]
    valid = np.arange(ocount)[None, :, None] < lens[:, None, None]
    return np.where(valid, x, x[:, :1, :])


def _dmajor16(x):
    """(B, W, D) fp32 -> bf16 [D, W*B] with column index w*B + c."""
    b, w, d = x.shape
    t = np.ascontiguousarray(x.transpose(1, 0, 2).reshape(w * b, d).T)
    return t.astype(ml_dtypes.bfloat16)


def kernel(im, im_l, s, s_l, pred, pred_l, cap_o_pred, cap_o_l, c_r_pred,
           c_r_l, trace=False, tmpdir=None):
    global LAST_RESULT, _NC
    im = np.asarray(im, dtype=np.float32)
    s = np.asarray(s, dtype=np.float32)
    pred = np.asarray(pred, dtype=np.float32)
    c_r_pred = np.asarray(c_r_pred, dtype=np.float32)
    im_l = np.asarray(im_l)
    pred_l = np.asarray(pred_l)

    im_p = _pad_objects(im, im_l)
    pred_p = _pad_objects(pred, pred_l)

    capT1 = _dmajor16(s)
    capT2 = _dmajor16(c_r_pred)
    rblob = np.stack([1.0 / np.asarray(s_l, dtype=np.float32),
                      1.0 / np.asarray(c_r_l, dtype=np.float32)], axis=1)

    in_maps = []
    for m in range(NCORES):
        sl = slice(m * IPC, (m + 1) * IPC)
        in_maps.append({
            "capT1": capT1,
            "capT2": capT2,
            "imT1": np.ascontiguousarray(
                im_p[sl].reshape(IPC * O1, D).T).astype(ml_dtypes.bfloat16),
            "imT2": np.ascontiguousarray(
                pred_p[sl].reshape(IPC * O2, D).T).astype(ml_dtypes.bfloat16),
            "rblob": rblob,
        })

    if _NC is None:
        _NC = _build_nc()
    res = run_bass_kernel_spmd(_NC, in_maps, list(range(NCORES)), trace=trace,
                               tmpdir=tmpdir)
    LAST_RESULT = res

    # scores_t[m] is [128 captions, 16 images] for images m*16..m*16+15.
    scores = np.concatenate(
        [res.results[m]["scores_t"].T for m in range(NCORES)], axis=0)

    # Triplet margin loss on the full (tiny) B x B matrix.
    d = np.diag(scores).copy()
    cost_s = np.maximum(MARGIN + scores - d[:, None], 0.0).astype(np.float32)
    cost_im = np.maximum(MARGIN + scores - d[None, :], 0.0).astype(np.float32)
    np.fill_diagonal(cost_s, 0.0)
    np.fill_diagonal(cost_im, 0.0)
    out = cost_s.max(axis=1).sum() + cost_im.max(axis=0).sum()
    return np.asarray(out, dtype=np.float32)


# revision 45
# speedup vs baseline: 1.6287x; 1.2258x over previous
"""Trainium2 Bass kernel for nn_ContrastiveLoss_66030827208766.

Strategy (data-parallel over images, captions replicated):
  - 8 cores, 16 images each.  Images are assigned to cores by GLOBAL
    length rank (core = rank % 8, slot = rank // 8), so every core's
    slot-k image has nearly the same valid-object count.  Only valid
    objects are shipped, padded per slot-group to a shared width: group
    A = slots 0-7 padded to Wa = len_sorted[63], group B = slots 8-15
    padded to Wb = len_sorted[127].  One program serves all cores.
  - Padding replicates object 0 (always valid), so a plain max over the
    padded block equals the masked max over valid objects.
  - All matmul operands are bf16 (PE accumulates fp32; end-to-end loss
    error ~1e-5).  Captions are replicated to every core in D-major
    layout [D, w*128 + c]: each 128-column slice is one caption word
    across all 128 captions.
  - Device per core: per caption word w, one matmul (stationary caption
    chunk [D,128], moving packed image-objects [D, C]) -> PSUM bank;
    grouped strided reduce_max over each slot's object block ->
    buf[c, w, slot]; reduce_sum over w; scale by 1/caption_len ->
    two [128 caps, 16 slots] tiles (parts sort by different keys) ->
    DRAM.
  - Host: unpermute slots of each part, add, then the (tiny) triplet
    margin loss reduction in numpy.

Codegen constraint: every TPB instruction can carry at most ONE sync
wait.  Three tactics keep us within it: (1) freshly-DMA'd tiles are
first touched by degenerate 1x1 "junk" matmuls so the real matmuls'
DMA-queue requirements are already observed by the PE; (2) buffers are
laid out so each writer hits a disjoint range (no spurious WAW chains);
(3) a post-pass strips waits that are redundant by construction
(same-engine in-order completion, per-queue DMA FIFO, barrier-covered
drain waits).
"""

import ml_dtypes
import numpy as np

import concourse.bass as bass
import concourse.mybir as mybir
from concourse import tile
from concourse.bass_utils import run_bass_kernel_spmd
from concourse.tile_rust import add_dep_helper

B = 128          # batch (images == captions)
O1, W1 = 36, 50  # part 1: im objects, s words
O2, W2 = 25, 30  # part 2: pred objects, c_r words
D = 128
NCORES = 8
IPC = B // NCORES  # images (slots) per core
G = IPC // 2       # slots per width-group
MARGIN = 0.2
F32 = mybir.dt.float32
BF16 = mybir.dt.bfloat16

LAST_RESULT = None   # BassKernelResults of the most recent run (for test.py)
_NC = None           # cached program
_NC_KEY = None       # widths the cached program was built for


def _build_part(nc, pending, hoist, cap, imt, buf, ps_tiles, ps_cols,
                W, Wa, Wb, cap_piece_cols, cap_key):
    """Emit matmul + grouped-reduce stream for one t2i part.

    Chunk layout: if C = 8*(Wa+Wb) fits one PSUM bank, chunk j of a tile
    sits in bank j (group A at +0, group B at +8*Wa); otherwise each
    chunk takes two banks (A at +0, B at +512).
    """
    C = G * (Wa + Wb)
    if C <= 512:
        banks_per_chunk, offA, offB = 1, 0, G * Wa
    else:
        banks_per_chunk, offA, offB = 2, 0, 512
    wc_per_piece = cap_piece_cols // B

    w = 0
    t_idx = 0
    while w < W:
        ps = ps_tiles[t_idx % len(ps_tiles)]
        cap_chunks = ps_cols[t_idx % len(ps_tiles)] // (512 * banks_per_chunk)
        n = min(cap_chunks, W - w)
        t_idx += 1
        for j in range(n):
            pc = (w + j) // wc_per_piece
            hoist((cap_key, pc),
                  cap[:1, pc * cap_piece_cols:pc * cap_piece_cols + 1])
            cs = cap[:, (w + j) * B:(w + j + 1) * B]
            base = j * banks_per_chunk * 512
            if banks_per_chunk == 1:
                mm = nc.tensor.matmul(ps[:, base:base + C], cs, imt[:],
                                      start=True, stop=True)
                while pending:
                    add_dep_helper(mm.ins, pending.pop().ins, sync=False,
                                   reason="order matmul after wait-carrier")
            else:
                for off, w0, wid in ((offA, 0, Wa), (offB, G * Wa, Wb)):
                    mm = nc.tensor.matmul(
                        ps[:, base + off:base + off + G * wid], cs,
                        imt[:, w0:w0 + G * wid], start=True, stop=True)
                    while pending:
                        add_dep_helper(mm.ins, pending.pop().ins, sync=False,
                                       reason="order matmul after wait-carrier")
        # Two grouped reduces (uniform width within each) covering all n
        # chunks of this tile.
        stride = banks_per_chunk * 512
        v = ps[:, :n * stride].rearrange("p (c x) -> p c x", c=n)
        for off, wid, s0, s1 in ((offA, Wa, 0, G), (offB, Wb, G, IPC)):
            nc.vector.reduce_max(
                buf[:, w:w + n, s0:s1],
                v[:, :, off:off + G * wid].rearrange(
                    "p c (g o) -> p c g o", o=wid),
                axis=mybir.AxisListType.X,
            )
        w += n


def _build_nc(widths):
    (Wa1, Wb1, Wa2, Wb2) = widths
    nc = bass.Bass()
    C1 = G * (Wa1 + Wb1)
    C2 = G * (Wa2 + Wb2)
    capT1 = nc.dram_tensor("capT1", [D, B * W1], BF16, kind="ExternalInput")
    capT2 = nc.dram_tensor("capT2", [D, B * W2], BF16, kind="ExternalInput")
    imT1 = nc.dram_tensor("imT1", [D, C1], BF16, kind="ExternalInput")
    imT2 = nc.dram_tensor("imT2", [D, C2], BF16, kind="ExternalInput")
    rblob = nc.dram_tensor("rblob", [B, 2], F32, kind="ExternalInput")
    out_t = nc.dram_tensor("scores_t", [B, 2 * IPC], F32,
                           kind="ExternalOutput")

    with tile.TileContext(nc) as tc:
        with (
            tc.tile_pool(name="const", bufs=1) as cpool,
            tc.tile_pool(name="psum", bufs=1, space="PSUM") as pspool,
            tc.tile_pool(name="work", bufs=1) as wpool,
        ):
            # ---- input DMAs: 16 total, alternating the two HWDGE rings
            # (even index -> sync, odd -> scalar).  DMAHW bookkeeping
            # lanes are assigned by global round-robin, so each of the 8
            # lanes sees a single issuing engine -> per-lane FIFO holds
            # and own-lane waits are strippable.  The output DMA is
            # emission #16 -> lane 0 (sync), same engine as lane 0's
            # inputs.
            dma_idx = [0]

            def load(dst_ap, src_ap):
                eng = nc.sync if dma_idx[0] % 2 == 0 else nc.scalar
                dma_idx[0] += 1
                return eng.dma_start(dst_ap, src_ap)

            NP1 = 10  # cap1 pieces (5 w-chunks each)
            NP2 = 3   # cap2 pieces (10 w-chunks each)
            P1C = B * W1 // NP1
            P2C = B * W2 // NP2

            imt1 = cpool.tile([D, C1], BF16, tag="imt1")
            load(imt1[:], imT1[:])
            cap1 = cpool.tile([D, B * W1], BF16, tag="cap1")
            for j in range(NP1):
                load(cap1[:, j * P1C:(j + 1) * P1C],
                     capT1[:, j * P1C:(j + 1) * P1C])
            imt2 = cpool.tile([D, C2], BF16, tag="imt2")
            load(imt2[:], imT2[:])
            cap2 = cpool.tile([D, B * W2], BF16, tag="cap2")
            for j in range(NP2):
                load(cap2[:, j * P2C:(j + 1) * P2C],
                     capT2[:, j * P2C:(j + 1) * P2C])
            rblob_sb = cpool.tile([B, 2], F32, tag="rblob")
            load(rblob_sb[:], rblob[:])
            r1 = rblob_sb[:, 0:1]
            r2 = rblob_sb[:, 1:2]
            assert dma_idx[0] == 16, dma_idx

            # w-major: each reduce writes a disjoint contiguous-ish range.
            buf1 = wpool.tile([B, W1, IPC], F32, tag="buf1")
            buf2 = wpool.tile([B, W2, IPC], F32, tag="buf2")

            # Static PSUM: 4-bank + 3-bank ping-pong tiles shared by both
            # parts, 1 junk bank.  (Pool slot rotation would bundle both
            # accessor engines' release waits onto one matmul.)
            psA = pspool.tile([B, 2048], F32, tag="psA", name="psA")
            psB = pspool.tile([B, 1536], F32, tag="psB", name="psB")
            junk_ps = pspool.tile([1, 1], F32, tag="junk_ps", name="junk_ps")

            hoisted = {}
            pending = []

            def hoist(key, corner_ap):
                if key in hoisted:
                    return
                hoisted[key] = nc.tensor.matmul(
                    junk_ps[:, :], corner_ap, corner_ap,
                    start=True, stop=True, skip_group_check=True,
                )
                pending.append(hoisted[key])

            hoist(("imt1",), imt1[:1, :1])
            _build_part(nc, pending, hoist, cap1, imt1, buf1, [psA, psB],
                        [2048, 1536], W1, Wa1, Wb1, P1C, "cap1")
            hoist(("imt2",), imt2[:1, :1])
            _build_part(nc, pending, hoist, cap2, imt2, buf2, [psA, psB],
                        [2048, 1536], W2, Wa2, Wb2, P2C, "cap2")

            sout = wpool.tile([B, 2 * IPC], F32, tag="sout")
            s1 = wpool.tile([B, IPC], F32, tag="s1")
            s2 = wpool.tile([B, IPC], F32, tag="s2")
            nc.vector.reduce_sum(s1[:], buf1[:].rearrange("p w i -> p i w"),
                                 axis=mybir.AxisListType.X)
            nc.vector.reduce_sum(s2[:], buf2[:].rearrange("p w i -> p i w"),
                                 axis=mybir.AxisListType.X)
            nc.vector.tensor_scalar_mul(sout[:, :IPC], s1[:], r1)
            nc.vector.tensor_scalar_mul(sout[:, IPC:], s2[:], r2)
            out_dma = nc.sync.dma_start(out_t[:], sout[:])

    # ---- wait-strip post-pass ----------------------------------------
    # Walrus codegen accepts at most one sync wait per instruction;
    # remove waits that are redundant by construction.
    out_q = {u.ant_name for u in out_dma.ins.sync_info.on_update
             if u.ant_name.startswith("DMAHW")}
    for bb in nc.main_func.blocks:
        for ins in bb.instructions:
            si = ins.sync_info
            if si is None:
                continue
            t = type(ins).__name__
            if t == "InstDrain" and len(si.on_wait) > 2:
                # Kernel-tail drain: engine completion is enforced by the
                # per-engine drains + EVSEM butterfly that follow, and
                # input-DMA completions are covered transitively by the
                # compute that consumed them.  Only the output DMA's
                # queue wait is load-bearing.
                drop = lambda w: w.ant_name not in out_q
            elif t == "InstMatmult":
                # WAW on a reused psum bank: the prior matmul's drain
                # (~128 cyc) finished >=2 matmul-streams earlier, so the
                # same-engine completion wait is dead.
                drop = lambda w: w.ant_name.startswith("PE_")
            elif getattr(ins, "engine", None) == mybir.EngineType.DVE:
                # DVE fully drains its pipe between ops; waits on earlier
                # DVE completions are satisfied at issue.
                drop = lambda w: w.ant_name.startswith("DVE_")
            elif t == "InstDMACopy":
                # Per-lane FIFO (single issuing engine per lane by
                # construction) makes own-lane waits redundant.
                own = {u.ant_name for u in si.on_update
                       if u.ant_name.startswith("DMAHW")}
                drop = lambda w: w.ant_name in own
            else:
                continue
            kept = [w for w in si.on_wait if not drop(w)]
            if len(kept) != len(si.on_wait):
                si.on_wait = kept
                ins.sync_info = si
    return nc


def _plan(lens, omax):
    """Global length-rank plan: order[r] = image of rank r; core r%8 slot
    r//8.  Group widths: Wa covers slots 0..G-1 (ranks < 64), Wb the
    rest."""
    lens = np.clip(np.asarray(lens, dtype=np.int64), 1, omax)
    order = np.argsort(lens, kind="stable")
    Wa = int(lens[order[NCORES * G - 1]])
    Wb = int(lens[order[B - 1]])
    return order, Wa, Wb


def _pack_images(x_bf, lens, order, Wa, Wb, core):
    """Build the packed, padded, D-major [D, G*(Wa+Wb)] bf16 image-object
    matrix for one core.  Slot k = image order[8k + core]; its first
    lens[i] objects, padded to the group width by replicating object 0."""
    cols = []
    for k in range(IPC):
        i = order[NCORES * k + core]
        wid = Wa if k < G else Wb
        L = min(int(lens[i]), wid)
        blk = np.empty((wid, D), dtype=x_bf.dtype)
        blk[:L] = x_bf[i, :L]
        blk[L:] = x_bf[i, 0]
        cols.append(blk)
    return np.ascontiguousarray(np.concatenate(cols, axis=0).T)


def kernel(im, im_l, s, s_l, pred, pred_l, cap_o_pred, cap_o_l, c_r_pred,
           c_r_l, trace=False, tmpdir=None):
    global LAST_RESULT, _NC, _NC_KEY
    im = np.asarray(im, dtype=np.float32)
    s = np.asarray(s, dtype=np.float32)
    pred = np.asarray(pred, dtype=np.float32)
    c_r_pred = np.asarray(c_r_pred, dtype=np.float32)
    im_l = np.asarray(im_l)
    pred_l = np.asarray(pred_l)

    order1, Wa1, Wb1 = _plan(im_l, O1)
    order2, Wa2, Wb2 = _plan(pred_l, O2)
    widths = (Wa1, Wb1, Wa2, Wb2)

    im_bf = im.astype(ml_dtypes.bfloat16)
    pred_bf = pred.astype(ml_dtypes.bfloat16)

    def dmajor16(x):
        b, w, d = x.shape
        t = np.ascontiguousarray(x.transpose(1, 0, 2).reshape(w * b, d).T)
        return t.astype(ml_dtypes.bfloat16)

    capT1 = dmajor16(s)
    capT2 = dmajor16(c_r_pred)
    rblob = np.stack([1.0 / np.asarray(s_l, dtype=np.float32),
                      1.0 / np.asarray(c_r_l, dtype=np.float32)], axis=1)

    in_maps = []
    for m in range(NCORES):
        in_maps.append({
            "capT1": capT1,
            "capT2": capT2,
            "imT1": _pack_images(im_bf, im_l, order1, Wa1, Wb1, m),
            "imT2": _pack_images(pred_bf, pred_l, order2, Wa2, Wb2, m),
            "rblob": rblob,
        })

    if _NC is None or _NC_KEY != widths:
        _NC = _build_nc(widths)
        _NC_KEY = widths
    res = run_bass_kernel_spmd(_NC, in_maps, list(range(NCORES)), trace=trace,
                               tmpdir=tmpdir)
    LAST_RESULT = res

    # Each core returns [128 caps, 32]: part-1 slots then part-2 slots,
    # already scaled by 1/caption_len.  Unpermute slots back to image
    # order and add the parts.
    scores = np.zeros((B, B), dtype=np.float32)
    for m in range(NCORES):
        tile_m = res.results[m]["scores_t"]  # [128, 32]
        idx1 = order1[np.arange(IPC) * NCORES + m]
        idx2 = order2[np.arange(IPC) * NCORES + m]
        scores[idx1, :] += tile_m[:, :IPC].T
        scores[idx2, :] += tile_m[:, IPC:].T

    # Triplet margin loss on the full (tiny) B x B matrix.
    d = np.diag(scores).copy()
    cost_s = np.maximum(MARGIN + scores - d[:, None], 0.0).astype(np.float32)
    cost_im = np.maximum(MARGIN + scores - d[None, :], 0.0).astype(np.float32)
    np.fill_diagonal(cost_s, 0.0)
    np.fill_diagonal(cost_im, 0.0)
    out = cost_s.max(axis=1).sum() + cost_im.max(axis=0).sum()
    return np.asarray(out, dtype=np.float32)


# revision 49
# speedup vs baseline: 1.6541x; 1.0156x over previous
"""Trainium2 Bass kernel for nn_ContrastiveLoss_66030827208766.

Strategy (data-parallel over images, captions replicated):
  - 8 cores, 16 images each.  Images are assigned to cores by GLOBAL
    length rank (core = rank % 8, slot = rank // 8), so every core's
    slot-k image has nearly the same valid-object count.  Only valid
    objects are shipped, padded per slot-group to a shared width: group
    A = slots 0-7 padded to Wa = len_sorted[63], group B = slots 8-15
    padded to Wb = len_sorted[127].  One program serves all cores.
  - Padding replicates object 0 (always valid), so a plain max over the
    padded block equals the masked max over valid objects.
  - All matmul operands are bf16 (PE accumulates fp32; end-to-end loss
    error ~1e-5).  Captions are replicated to every core in D-major
    layout [D, w*128 + c]: each 128-column slice is one caption word
    across all 128 captions.
  - Device per core: per caption word w, one matmul (stationary caption
    chunk [D,128], moving packed image-objects [D, C]) -> PSUM bank;
    grouped strided reduce_max over each slot's object block ->
    buf[c, w, slot]; reduce_sum over w; scale by 1/caption_len ->
    two [128 caps, 16 slots] tiles (parts sort by different keys) ->
    DRAM.
  - Host: unpermute slots of each part, add, then the (tiny) triplet
    margin loss reduction in numpy.

Codegen constraint: every TPB instruction can carry at most ONE sync
wait.  Three tactics keep us within it: (1) freshly-DMA'd tiles are
first touched by degenerate 1x1 "junk" matmuls so the real matmuls'
DMA-queue requirements are already observed by the PE; (2) buffers are
laid out so each writer hits a disjoint range (no spurious WAW chains);
(3) a post-pass strips waits that are redundant by construction
(same-engine in-order completion, per-queue DMA FIFO, barrier-covered
drain waits).
"""

import ml_dtypes
import numpy as np

import concourse.bass as bass
import concourse.mybir as mybir
from concourse import tile
from concourse.bass_utils import run_bass_kernel_spmd
from concourse.tile_rust import add_dep_helper

B = 128          # batch (images == captions)
O1, W1 = 36, 50  # part 1: im objects, s words
O2, W2 = 25, 30  # part 2: pred objects, c_r words
D = 128
NCORES = 8
IPC = B // NCORES  # images (slots) per core
G = IPC // 2       # slots per width-group
MARGIN = 0.2
F32 = mybir.dt.float32
BF16 = mybir.dt.bfloat16

LAST_RESULT = None   # BassKernelResults of the most recent run (for test.py)
_NC = None           # cached program
_NC_KEY = None       # widths the cached program was built for


def _build_part(nc, pending, hoist, cap, imt, buf, ps_tiles, ps_cols,
                W, Wa, Wb, cap_piece_cols, cap_key):
    """Emit matmul + grouped-reduce stream for one t2i part.

    Chunk layout: if C = 8*(Wa+Wb) fits one PSUM bank, chunk j of a tile
    sits in bank j (group A at +0, group B at +8*Wa); otherwise each
    chunk takes two banks (A at +0, B at +512).
    """
    C = G * (Wa + Wb)
    if C <= 512:
        banks_per_chunk, offA, offB = 1, 0, G * Wa
    else:
        banks_per_chunk, offA, offB = 2, 0, 512
    wc_per_piece = cap_piece_cols // B

    w = 0
    t_idx = 0
    while w < W:
        ps = ps_tiles[t_idx % len(ps_tiles)]
        cap_chunks = ps_cols[t_idx % len(ps_tiles)] // (512 * banks_per_chunk)
        n = min(cap_chunks, W - w)
        t_idx += 1
        for j in range(n):
            pc = (w + j) // wc_per_piece
            hoist((cap_key, pc),
                  cap[:1, pc * cap_piece_cols:pc * cap_piece_cols + 1])
            cs = cap[:, (w + j) * B:(w + j + 1) * B]
            base = j * banks_per_chunk * 512
            if banks_per_chunk == 1:
                mm = nc.tensor.matmul(ps[:, base:base + C], cs, imt[:],
                                      start=True, stop=True)
                while pending:
                    add_dep_helper(mm.ins, pending.pop().ins, sync=False,
                                   reason="order matmul after wait-carrier")
            else:
                for off, w0, wid in ((offA, 0, Wa), (offB, G * Wa, Wb)):
                    mm = nc.tensor.matmul(
                        ps[:, base + off:base + off + G * wid], cs,
                        imt[:, w0:w0 + G * wid], start=True, stop=True)
                    while pending:
                        add_dep_helper(mm.ins, pending.pop().ins, sync=False,
                                       reason="order matmul after wait-carrier")
        # Two grouped reduces (uniform width within each) covering all n
        # chunks of this tile.
        stride = banks_per_chunk * 512
        v = ps[:, :n * stride].rearrange("p (c x) -> p c x", c=n)
        for off, wid, s0, s1 in ((offA, Wa, 0, G), (offB, Wb, G, IPC)):
            nc.vector.reduce_max(
                buf[:, w:w + n, s0:s1],
                v[:, :, off:off + G * wid].rearrange(
                    "p c (g o) -> p c g o", o=wid),
                axis=mybir.AxisListType.X,
            )
        w += n


def _build_nc(widths):
    (Wa1, Wb1, Wa2, Wb2) = widths
    nc = bass.Bass()
    C1 = G * (Wa1 + Wb1)
    C2 = G * (Wa2 + Wb2)
    capT1 = nc.dram_tensor("capT1", [D, B * W1], BF16, kind="ExternalInput")
    capT2 = nc.dram_tensor("capT2", [D, B * W2], BF16, kind="ExternalInput")
    imT1 = nc.dram_tensor("imT1", [D, C1], BF16, kind="ExternalInput")
    imT2 = nc.dram_tensor("imT2", [D, C2], BF16, kind="ExternalInput")
    rblob = nc.dram_tensor("rblob", [B, 2], F32, kind="ExternalInput")
    out_t = nc.dram_tensor("scores_t", [B, 2 * IPC], F32,
                           kind="ExternalOutput")

    with tile.TileContext(nc) as tc:
        with (
            tc.tile_pool(name="const", bufs=1) as cpool,
            tc.tile_pool(name="psum", bufs=1, space="PSUM") as pspool,
            tc.tile_pool(name="work", bufs=1) as wpool,
        ):
            # ---- input DMAs: 16 total, alternating the two HWDGE rings
            # (even index -> sync, odd -> scalar).  DMAHW bookkeeping
            # lanes are assigned by global round-robin, so each of the 8
            # lanes sees a single issuing engine -> per-lane FIFO holds
            # and own-lane waits are strippable.  The output DMA is
            # emission #16 -> lane 0 (sync), same engine as lane 0's
            # inputs.
            dma_idx = [0]

            def load(dst_ap, src_ap):
                eng = nc.sync if dma_idx[0] % 2 == 0 else nc.scalar
                dma_idx[0] += 1
                return eng.dma_start(dst_ap, src_ap)

            NP1 = 10  # cap1 pieces (5 w-chunks each): early words land early
            NP2 = 1   # cap2: one DMA, only needed after part 1 finishes
            P1C = B * W1 // NP1
            P2C = B * W2 // NP2

            imt1 = cpool.tile([D, C1], BF16, tag="imt1")
            load(imt1[:], imT1[:])
            cap1 = cpool.tile([D, B * W1], BF16, tag="cap1")
            for j in range(NP1):
                load(cap1[:, j * P1C:(j + 1) * P1C],
                     capT1[:, j * P1C:(j + 1) * P1C])
            imt2 = cpool.tile([D, C2], BF16, tag="imt2")
            load(imt2[:], imT2[:])
            cap2 = cpool.tile([D, B * W2], BF16, tag="cap2")
            for j in range(NP2):
                load(cap2[:, j * P2C:(j + 1) * P2C],
                     capT2[:, j * P2C:(j + 1) * P2C])
            rblob_sb = cpool.tile([B, 2], F32, tag="rblob")
            load(rblob_sb[:], rblob[:])
            r1 = rblob_sb[:, 0:1]
            r2 = rblob_sb[:, 1:2]
            # 14 input DMAs; the output DMA is emission #14 -> lane 6,
            # whose earlier user (emission #6) is also sync-issued.
            assert dma_idx[0] == 14, dma_idx

            # w-major: each reduce writes a disjoint contiguous-ish range.
            buf1 = wpool.tile([B, W1, IPC], F32, tag="buf1")
            buf2 = wpool.tile([B, W2, IPC], F32, tag="buf2")

            # Static PSUM: 4-bank + 3-bank ping-pong tiles shared by both
            # parts, 1 junk bank.  (Pool slot rotation would bundle both
            # accessor engines' release waits onto one matmul.)
            psA = pspool.tile([B, 2048], F32, tag="psA", name="psA")
            psB = pspool.tile([B, 1536], F32, tag="psB", name="psB")
            junk_ps = pspool.tile([1, 1], F32, tag="junk_ps", name="junk_ps")

            hoisted = {}
            pending = []

            def hoist(key, corner_ap):
                if key in hoisted:
                    return
                hoisted[key] = nc.tensor.matmul(
                    junk_ps[:, :], corner_ap, corner_ap,
                    start=True, stop=True, skip_group_check=True,
                )
                pending.append(hoisted[key])

            sout = wpool.tile([B, 2 * IPC], F32, tag="sout")
            s1 = wpool.tile([B, IPC], F32, tag="s1")
            s2 = wpool.tile([B, IPC], F32, tag="s2")

            hoist(("imt1",), imt1[:1, :1])
            _build_part(nc, pending, hoist, cap1, imt1, buf1, [psA, psB],
                        [2048, 1536], W1, Wa1, Wb1, P1C, "cap1")
            # Part-1 epilogue emitted before part 2: the DVE executes its
            # queue in order, so this overlaps part-2 matmuls.
            nc.vector.reduce_sum(s1[:], buf1[:].rearrange("p w i -> p i w"),
                                 axis=mybir.AxisListType.X)
            nc.vector.tensor_scalar_mul(sout[:, :IPC], s1[:], r1)

            hoist(("imt2",), imt2[:1, :1])
            _build_part(nc, pending, hoist, cap2, imt2, buf2, [psA, psB],
                        [2048, 1536], W2, Wa2, Wb2, P2C, "cap2")
            nc.vector.reduce_sum(s2[:], buf2[:].rearrange("p w i -> p i w"),
                                 axis=mybir.AxisListType.X)
            nc.vector.tensor_scalar_mul(sout[:, IPC:], s2[:], r2)
            out_dma = nc.sync.dma_start(out_t[:], sout[:])

    # ---- wait-strip post-pass ----------------------------------------
    # Walrus codegen accepts at most one sync wait per instruction;
    # remove waits that are redundant by construction.
    out_q = {u.ant_name for u in out_dma.ins.sync_info.on_update
             if u.ant_name.startswith("DMAHW")}
    for bb in nc.main_func.blocks:
        for ins in bb.instructions:
            si = ins.sync_info
            if si is None:
                continue
            t = type(ins).__name__
            if t == "InstDrain" and len(si.on_wait) > 2:
                # Kernel-tail drain: engine completion is enforced by the
                # per-engine drains + EVSEM butterfly that follow, and
                # input-DMA completions are covered transitively by the
                # compute that consumed them.  Only the output DMA's
                # queue wait is load-bearing.
                drop = lambda w: w.ant_name not in out_q
            elif t == "InstMatmult":
                # WAW on a reused psum bank: the prior matmul's drain
                # (~128 cyc) finished >=2 matmul-streams earlier, so the
                # same-engine completion wait is dead.
                drop = lambda w: w.ant_name.startswith("PE_")
            elif getattr(ins, "engine", None) == mybir.EngineType.DVE:
                # DVE fully drains its pipe between ops; waits on earlier
                # DVE completions are satisfied at issue.
                drop = lambda w: w.ant_name.startswith("DVE_")
            elif t == "InstDMACopy":
                # Per-lane FIFO (single issuing engine per lane by
                # construction) makes own-lane waits redundant.
                own = {u.ant_name for u in si.on_update
                       if u.ant_name.startswith("DMAHW")}
                drop = lambda w: w.ant_name in own
            else:
                continue
            kept = [w for w in si.on_wait if not drop(w)]
            if len(kept) != len(si.on_wait):
                si.on_wait = kept
                ins.sync_info = si
    return nc


def _plan(lens, omax):
    """Global length-rank plan: order[r] = image of rank r; core r%8 slot
    r//8.  Group widths: Wa covers slots 0..G-1 (ranks < 64), Wb the
    rest."""
    lens = np.clip(np.asarray(lens, dtype=np.int64), 1, omax)
    order = np.argsort(lens, kind="stable")
    Wa = int(lens[order[NCORES * G - 1]])
    Wb = int(lens[order[B - 1]])
    return order, Wa, Wb


def _pack_images(x_bf, lens, order, Wa, Wb, core):
    """Build the packed, padded, D-major [D, G*(Wa+Wb)] bf16 image-object
    matrix for one core.  Slot k = image order[8k + core]; its first
    lens[i] objects, padded to the group width by replicating object 0."""
    cols = []
    for k in range(IPC):
        i = order[NCORES * k + core]
        wid = Wa if k < G else Wb
        L = min(int(lens[i]), wid)
        blk = np.empty((wid, D), dtype=x_bf.dtype)
        blk[:L] = x_bf[i, :L]
        blk[L:] = x_bf[i, 0]
        cols.append(blk)
    return np.ascontiguousarray(np.concatenate(cols, axis=0).T)


def kernel(im, im_l, s, s_l, pred, pred_l, cap_o_pred, cap_o_l, c_r_pred,
           c_r_l, trace=False, tmpdir=None):
    global LAST_RESULT, _NC, _NC_KEY
    im = np.asarray(im, dtype=np.float32)
    s = np.asarray(s, dtype=np.float32)
    pred = np.asarray(pred, dtype=np.float32)
    c_r_pred = np.asarray(c_r_pred, dtype=np.float32)
    im_l = np.asarray(im_l)
    pred_l = np.asarray(pred_l)

    order1, Wa1, Wb1 = _plan(im_l, O1)
    order2, Wa2, Wb2 = _plan(pred_l, O2)
    widths = (Wa1, Wb1, Wa2, Wb2)

    im_bf = im.astype(ml_dtypes.bfloat16)
    pred_bf = pred.astype(ml_dtypes.bfloat16)

    def dmajor16(x):
        b, w, d = x.shape
        t = np.ascontiguousarray(x.transpose(1, 0, 2).reshape(w * b, d).T)
        return t.astype(ml_dtypes.bfloat16)

    capT1 = dmajor16(s)
    capT2 = dmajor16(c_r_pred)
    rblob = np.stack([1.0 / np.asarray(s_l, dtype=np.float32),
                      1.0 / np.asarray(c_r_l, dtype=np.float32)], axis=1)

    in_maps = []
    for m in range(NCORES):
        in_maps.append({
            "capT1": capT1,
            "capT2": capT2,
            "imT1": _pack_images(im_bf, im_l, order1, Wa1, Wb1, m),
            "imT2": _pack_images(pred_bf, pred_l, order2, Wa2, Wb2, m),
            "rblob": rblob,
        })

    if _NC is None or _NC_KEY != widths:
        _NC = _build_nc(widths)
        _NC_KEY = widths
    res = run_bass_kernel_spmd(_NC, in_maps, list(range(NCORES)), trace=trace,
                               tmpdir=tmpdir)
    LAST_RESULT = res

    # Each core returns [128 caps, 32]: part-1 slots then part-2 slots,
    # already scaled by 1/caption_len.  Unpermute slots back to image
    # order and add the parts.
    scores = np.zeros((B, B), dtype=np.float32)
    for m in range(NCORES):
        tile_m = res.results[m]["scores_t"]  # [128, 32]
        idx1 = order1[np.arange(IPC) * NCORES + m]
        idx2 = order2[np.arange(IPC) * NCORES + m]
        scores[idx1, :] += tile_m[:, :IPC].T
        scores[idx2, :] += tile_m[:, IPC:].T

    # Triplet margin loss on the full (tiny) B x B matrix.
    d = np.diag(scores).copy()
    cost_s = np.maximum(MARGIN + scores - d[:, None], 0.0).astype(np.float32)
    cost_im = np.maximum(MARGIN + scores - d[None, :], 0.0).astype(np.float32)
    np.fill_diagonal(cost_s, 0.0)
    np.fill_diagonal(cost_im, 0.0)
    out = cost_s.max(axis=1).sum() + cost_im.max(axis=0).sum()
    return np.asarray(out, dtype=np.float32)
